# revision 23
# baseline (speedup 1.0000x reference)
"""Trainium2 Bass kernel for nn_AdditionalTermLayer (focal/tail-weighted CE penalty).

Strategy (data-parallel over the batch, 8 NeuronCores):
  - Each core streams its [2048, 8192] f32 shard of `inputs` through SBUF once,
    in 128-row tiles, each loaded as NSPLIT column chunks so DVE/ACT compute on
    chunk j overlaps the DMA of chunk j+1 (DMA is the roofline: ~64 MiB/core).
  - Per tile: row max (DVE chunk reduces + combine), sum(exp(x)) (ACT exp with
    accum_out; no max shift needed — the host guard bounds |x|), and per-row
    `x[:, C-16:] >= rowmax` compares (DVE) whose sums give the argmax-count
    histogram restricted to the 16 tail classes (the only classes whose counts
    the loss actually needs — this avoids computing argmax positions entirely).
  - Host combines the tiny per-core outputs: gathers x[b, label], computes the
    per-sample penalty, the tail-class histogram all-reduce, adaptive weights,
    and the final scalar mean.
"""

import sys
import types

import numpy as np


def _ensure_ntff_hook():
    """The axon boot registers its NTFF profile hook only if
    `antenv.axon_hooks` exists; on images where it doesn't, bass_utils
    crashes importing it under BASS_TRACE. Provide the module and register
    the ctypes-based hook ourselves so profiling works."""
    try:
        import antenv.axon_hooks  # noqa: F401
        return
    except ImportError:
        pass
    mod = types.ModuleType("antenv.axon_hooks")
    mod._hook = None

    def set_axon_ntff_profile_hook(h):
        mod._hook = h

    def get_axon_ntff_profile_hook():
        return mod._hook

    mod.set_axon_ntff_profile_hook = set_axon_ntff_profile_hook
    mod.get_axon_ntff_profile_hook = get_axon_ntff_profile_hook
    sys.modules["antenv.axon_hooks"] = mod
    try:
        import antenv
        antenv.axon_hooks = mod
    except ImportError:
        pass
    try:
        from trn_agent_boot.trn_boot import _ntff_profile_via_ctypes
        hook = _ntff_profile_via_ctypes("/opt/axon/libaxon_pjrt.so")
        if hook is not None:
            set_axon_ntff_profile_hook(hook)
    except Exception:
        pass


_ensure_ntff_hook()

import concourse.tile as tile
from concourse import bacc, mybir
from concourse.bass_utils import run_bass_kernel_spmd

B = 16384
C = 8192
N_CORES = 8
RPC = B // N_CORES  # rows per core = 2048
P = 128             # SBUF partitions
T = RPC // P        # tiles per core = 16
NTAIL = 16          # classes whose argmax-counts we need (last 16)

F32 = mybir.dt.float32
NSPLIT = 4       # column chunks per tile (pipeline granularity)

_COMPILED_NC = None
LAST_RESULTS = None  # test harness reads exec_time_ns from here


def _build_nc(nsplit=NSPLIT):
    nc = bacc.Bacc(
        "TRN2",
        target_bir_lowering=False,
        debug=False,
        num_devices=N_CORES,
    )
    W = C // nsplit
    x_ext = nc.dram_tensor("x", [RPC, C], F32, kind="ExternalInput")
    s_ext = nc.dram_tensor("s_out", [P, nsplit * T], F32, kind="ExternalOutput")
    cnt_ext = nc.dram_tensor("cnt_out", [P, NTAIL], F32, kind="ExternalOutput")

    with tile.TileContext(nc) as tc:
        with (
            tc.tile_pool(name="xin", bufs=5) as xin_pool,
            tc.tile_pool(name="stats", bufs=1) as stats_pool,
            tc.tile_pool(name="dump", bufs=1) as dump_pool,
        ):
            m_all = stats_pool.tile([P, T], F32, tag="m_all")
            s_all = stats_pool.tile([P, nsplit * T], F32, tag="s_all")
            ge_all = stats_pool.tile([P, NTAIL, T], F32, tag="ge_all")
            cnt = stats_pool.tile([P, NTAIL], F32, tag="cnt")
            dump = dump_pool.tile([P, W], F32, tag="dump")

            for t in range(T):
                xt = xin_pool.tile([P, C], F32, tag="xt")
                rows = slice(t * P, (t + 1) * P)
                mh = xin_pool.tile([P, nsplit], F32, tag="mh")
                for j in range(nsplit):
                    cols = slice(j * W, (j + 1) * W)
                    # chunked DMAs so compute on chunk j overlaps the load of j+1
                    nc.sync.dma_start(out=xt[:, cols], in_=x_ext[rows, cols])
                    # chunk max (feeds only the tail-class compares)
                    nc.vector.tensor_reduce(
                        out=mh[:, j:j + 1], in_=xt[:, cols],
                        axis=mybir.AxisListType.X, op=mybir.AluOpType.max,
                    )
                    # sum(exp(x)) via ACT accumulate; elementwise out is
                    # discarded. No max subtraction: |x| <= 60 is guaranteed by
                    # the host-side guard, so exp can't overflow f32.
                    nc.scalar.activation(
                        out=dump[:],
                        in_=xt[:, cols],
                        func=mybir.ActivationFunctionType.Exp,
                        bias=0.0,
                        scale=1.0,
                        accum_out=s_all[:, nsplit * t + j:nsplit * t + j + 1],
                    )
                nc.vector.tensor_reduce(
                    out=m_all[:, t:t + 1], in_=mh[:],
                    axis=mybir.AxisListType.X, op=mybir.AluOpType.max,
                )

                # tail-class hit mask: x[:, C-16:] >= rowmax  (1.0 iff argmax)
                nc.vector.tensor_scalar(
                    ge_all[:, :, t:t + 1],
                    xt[:, C - NTAIL:C],
                    m_all[:, t:t + 1],
                    None,
                    mybir.AluOpType.is_ge,
                )

            # per-partition tail counts summed over the 16 tiles
            nc.vector.tensor_reduce(
                out=cnt[:],
                in_=ge_all[:],
                axis=mybir.AxisListType.X,
                op=mybir.AluOpType.add,
            )

            nc.sync.dma_start(out=s_ext[:, :], in_=s_all[:])
            nc.sync.dma_start(out=cnt_ext[:, :], in_=cnt[:])

    nc.compile()
    return nc


def _get_nc():
    global _COMPILED_NC
    if _COMPILED_NC is None:
        _COMPILED_NC = _build_nc()
    return _COMPILED_NC


def _host_reference(x, true_labels, prev_counts, tail_mask):
    """Pure-numpy fallback mirroring the reference; used only if tail_mask is
    not a subset of the last NTAIL classes (never expected for this problem)."""
    preds = np.argmax(x, axis=-1)
    curr_counts = np.bincount(preds, minlength=x.shape[1]).astype(np.float64)
    m = x.max(axis=-1)
    S = np.exp(x - m[:, None]).sum(axis=-1)
    xt = x[np.arange(x.shape[0]), true_labels]
    p = np.exp(xt - m - np.log(S))
    base = -np.log(p + 1e-7) * (1.0 - p)
    prev = prev_counts[true_labels].astype(np.float64)
    curr = curr_counts[true_labels]
    tail_w = np.where((prev > 0) & (curr < prev), 4.0,
                      np.where((prev > 0) & (curr > prev), 2.0, 3.0))
    w = np.where(tail_mask[true_labels], tail_w, 1.0)
    return np.array((base * w).mean() * 0.1, dtype=np.float32)


def kernel(inputs, true_labels, prev_counts, tail_mask):
    global LAST_RESULTS
    inputs = np.asarray(inputs, dtype=np.float32)
    true_labels = np.asarray(true_labels).astype(np.int64)
    prev_counts = np.asarray(prev_counts)
    tail_mask = np.asarray(tail_mask).astype(bool)
    assert inputs.shape == (B, C), inputs.shape

    if not np.isfinite(inputs).all():
        inputs = np.nan_to_num(inputs)

    tail_idx = np.flatnonzero(tail_mask)
    if (tail_idx.size and tail_idx.min() < C - NTAIL) or np.abs(inputs).max() > 60.0:
        # unexpected tail layout, or values large enough that the device's
        # unshifted exp could overflow -> use the exact host path
        return _host_reference(inputs, true_labels, prev_counts, tail_mask)

    in_maps = [{"x": inputs[i * RPC:(i + 1) * RPC]} for i in range(N_CORES)]
    res = None
    for attempt in range(2):
        try:
            nc = _get_nc()
            LAST_RESULTS = run_bass_kernel_spmd(
                nc, in_maps, core_ids=list(range(N_CORES))
            )
            res = LAST_RESULTS.results
            break
        except Exception:
            if attempt == 1:
                # device unavailable/wedged -> slow but exact host path
                return _host_reference(inputs, true_labels, prev_counts, tail_mask)

    # [P, T] per core, row = core*RPC + t*P + p  ->  transpose to [T, P] then flatten
    S = np.concatenate(
        [r["s_out"].reshape(P, T, NSPLIT).sum(-1).T.reshape(-1) for r in res]
    ).astype(np.float64)
    cnt_tail = np.sum([r["cnt_out"].sum(axis=0) for r in res], axis=0).astype(np.float64)

    xt = inputs[np.arange(B), true_labels].astype(np.float64)
    p = np.exp(xt - np.log(S))
    base = -np.log(p + 1e-7) * (1.0 - p)

    is_tail = tail_mask[true_labels]
    prev = prev_counts[true_labels].astype(np.float64)
    curr = np.zeros(B, dtype=np.float64)
    if is_tail.any():
        curr[is_tail] = cnt_tail[true_labels[is_tail] - (C - NTAIL)]
    tail_w = np.where((prev > 0) & (curr < prev), 4.0,
                      np.where((prev > 0) & (curr > prev), 2.0, 3.0))
    w = np.where(is_tail, tail_w, 1.0)

    return np.array((base * w).mean() * 0.1, dtype=np.float32)


# revision 27
# speedup vs baseline: 1.0001x; 1.0001x over previous
"""Trainium2 Bass kernel for nn_AdditionalTermLayer (focal/tail-weighted CE penalty).

Strategy (data-parallel over the batch, 8 NeuronCores):
  - Each core streams its [2048, 8192] f32 shard of `inputs` through SBUF once,
    in 128-row tiles, each loaded as NSPLIT column chunks so DVE/ACT compute on
    chunk j overlaps the DMA of chunk j+1 (DMA is the roofline: ~64 MiB/core).
  - Per tile: row max (DVE chunk reduces + combine), sum(exp(x)) (ACT exp with
    accum_out; no max shift needed — the host guard bounds |x|), and per-row
    `x[:, C-16:] >= rowmax` compares (DVE) whose sums give the argmax-count
    histogram restricted to the 16 tail classes (the only classes whose counts
    the loss actually needs — this avoids computing argmax positions entirely).
  - Host combines the tiny per-core outputs: gathers x[b, label], computes the
    per-sample penalty, the tail-class histogram all-reduce, adaptive weights,
    and the final scalar mean.
"""

import sys
import types

import numpy as np


def _ensure_ntff_hook():
    """The axon boot registers its NTFF profile hook only if
    `antenv.axon_hooks` exists; on images where it doesn't, bass_utils
    crashes importing it under BASS_TRACE. Provide the module and register
    the ctypes-based hook ourselves so profiling works."""
    try:
        import antenv.axon_hooks  # noqa: F401
        return
    except ImportError:
        pass
    mod = types.ModuleType("antenv.axon_hooks")
    mod._hook = None

    def set_axon_ntff_profile_hook(h):
        mod._hook = h

    def get_axon_ntff_profile_hook():
        return mod._hook

    mod.set_axon_ntff_profile_hook = set_axon_ntff_profile_hook
    mod.get_axon_ntff_profile_hook = get_axon_ntff_profile_hook
    sys.modules["antenv.axon_hooks"] = mod
    try:
        import antenv
        antenv.axon_hooks = mod
    except ImportError:
        pass
    try:
        from trn_agent_boot.trn_boot import _ntff_profile_via_ctypes
        hook = _ntff_profile_via_ctypes("/opt/axon/libaxon_pjrt.so")
        if hook is not None:
            set_axon_ntff_profile_hook(hook)
    except Exception:
        pass


_ensure_ntff_hook()

import concourse.tile as tile
from concourse import bacc, mybir
from concourse.bass_utils import run_bass_kernel_spmd

B = 16384
C = 8192
N_CORES = 8
RPC = B // N_CORES  # rows per core = 2048
P = 128             # SBUF partitions
T = RPC // P        # tiles per core = 16
NTAIL = 16          # classes whose argmax-counts we need (last 16)

F32 = mybir.dt.float32
NSPLIT = 4       # column chunks per tile (pipeline granularity)
TAPER = False    # shrink the last tile's trailing chunks to cut the drain tail
DUAL_RING_TILES = 0 # tiles whose chunk DMAs alternate Sync/Scalar HWDGE rings


def _chunk_plans(nsplit=NSPLIT, taper=TAPER):
    W = C // nsplit
    plans = [[W] * nsplit for _ in range(T)]
    if taper:
        plans[T - 1] = [2048, 2048, 2048, 1536, 512]
    assert all(sum(p) == C for p in plans)
    return plans

_COMPILED_NC = None
LAST_RESULTS = None  # test harness reads exec_time_ns from here


def _build_nc(nsplit=NSPLIT, taper=TAPER, dual_ring_tiles=DUAL_RING_TILES):
    nc = bacc.Bacc(
        "TRN2",
        target_bir_lowering=False,
        debug=False,
        num_devices=N_CORES,
    )
    plans = _chunk_plans(nsplit, taper)
    n_chunks = sum(len(p) for p in plans)
    max_w = max(w for p in plans for w in p)
    x_ext = nc.dram_tensor("x", [RPC, C], F32, kind="ExternalInput")
    s_ext = nc.dram_tensor("s_out", [P, n_chunks], F32, kind="ExternalOutput")
    cnt_ext = nc.dram_tensor("cnt_out", [P, NTAIL, T], F32, kind="ExternalOutput")

    with tile.TileContext(nc) as tc:
        with (
            tc.tile_pool(name="xin", bufs=5) as xin_pool,
            tc.tile_pool(name="stats", bufs=1) as stats_pool,
            tc.tile_pool(name="dump", bufs=1) as dump_pool,
        ):
            m_all = stats_pool.tile([P, T], F32, tag="m_all")
            s_all = stats_pool.tile([P, n_chunks], F32, tag="s_all")
            ge_all = stats_pool.tile([P, NTAIL, T], F32, tag="ge_all")
            dump = dump_pool.tile([P, max_w], F32, tag="dump")

            ci = 0  # global chunk index
            for t, plan in enumerate(plans):
                xt = xin_pool.tile([P, C], F32, tag="xt")
                rows = slice(t * P, (t + 1) * P)
                mh = xin_pool.tile([P, len(plan)], F32, tag="mh")
                col = 0
                for j, w in enumerate(plan):
                    cols = slice(col, col + w)
                    col += w
                    # chunked DMAs so compute on chunk j overlaps the load of
                    # j+1; during ramp-up alternate the two HWDGE rings
                    dma_eng = (
                        nc.scalar if (t < dual_ring_tiles and j % 2) else nc.sync
                    )
                    dma_eng.dma_start(out=xt[:, cols], in_=x_ext[rows, cols])
                    # chunk max (feeds only the tail-class compares)
                    nc.vector.tensor_reduce(
                        out=mh[:, j:j + 1], in_=xt[:, cols],
                        axis=mybir.AxisListType.X, op=mybir.AluOpType.max,
                    )
                    # sum(exp(x)) via ACT accumulate; elementwise out is
                    # discarded. No max subtraction: |x| <= 60 is guaranteed by
                    # the host-side guard, so exp can't overflow f32.
                    nc.scalar.activation(
                        out=dump[:, 0:w],
                        in_=xt[:, cols],
                        func=mybir.ActivationFunctionType.Exp,
                        bias=0.0,
                        scale=1.0,
                        accum_out=s_all[:, ci:ci + 1],
                    )
                    ci += 1
                nc.vector.tensor_reduce(
                    out=m_all[:, t:t + 1], in_=mh[:],
                    axis=mybir.AxisListType.X, op=mybir.AluOpType.max,
                )

                # tail-class hit mask: x[:, C-16:] >= rowmax  (1.0 iff argmax)
                nc.vector.tensor_scalar(
                    ge_all[:, :, t:t + 1],
                    xt[:, C - NTAIL:C],
                    m_all[:, t:t + 1],
                    None,
                    mybir.AluOpType.is_ge,
                )

            # raw per-tile masks go to the host (it sums them); no device
            # reduction in the drain-critical path
            nc.sync.dma_start(out=s_ext[:, :], in_=s_all[:])
            nc.sync.dma_start(out=cnt_ext[:, :, :], in_=ge_all[:])

    nc.compile()
    return nc


def _get_nc():
    global _COMPILED_NC
    if _COMPILED_NC is None:
        _COMPILED_NC = _build_nc()
    return _COMPILED_NC


def _host_reference(x, true_labels, prev_counts, tail_mask):
    """Pure-numpy fallback mirroring the reference; used only if tail_mask is
    not a subset of the last NTAIL classes (never expected for this problem)."""
    preds = np.argmax(x, axis=-1)
    curr_counts = np.bincount(preds, minlength=x.shape[1]).astype(np.float64)
    m = x.max(axis=-1)
    S = np.exp(x - m[:, None]).sum(axis=-1)
    xt = x[np.arange(x.shape[0]), true_labels]
    p = np.exp(xt - m - np.log(S))
    base = -np.log(p + 1e-7) * (1.0 - p)
    prev = prev_counts[true_labels].astype(np.float64)
    curr = curr_counts[true_labels]
    tail_w = np.where((prev > 0) & (curr < prev), 4.0,
                      np.where((prev > 0) & (curr > prev), 2.0, 3.0))
    w = np.where(tail_mask[true_labels], tail_w, 1.0)
    return np.array((base * w).mean() * 0.1, dtype=np.float32)


def kernel(inputs, true_labels, prev_counts, tail_mask):
    global LAST_RESULTS
    inputs = np.asarray(inputs, dtype=np.float32)
    true_labels = np.asarray(true_labels).astype(np.int64)
    prev_counts = np.asarray(prev_counts)
    tail_mask = np.asarray(tail_mask).astype(bool)
    assert inputs.shape == (B, C), inputs.shape

    if not np.isfinite(inputs).all():
        inputs = np.nan_to_num(inputs)

    tail_idx = np.flatnonzero(tail_mask)
    if (tail_idx.size and tail_idx.min() < C - NTAIL) or np.abs(inputs).max() > 60.0:
        # unexpected tail layout, or values large enough that the device's
        # unshifted exp could overflow -> use the exact host path
        return _host_reference(inputs, true_labels, prev_counts, tail_mask)

    in_maps = [{"x": inputs[i * RPC:(i + 1) * RPC]} for i in range(N_CORES)]
    res = None
    for attempt in range(2):
        try:
            nc = _get_nc()
            LAST_RESULTS = run_bass_kernel_spmd(
                nc, in_maps, core_ids=list(range(N_CORES))
            )
            res = LAST_RESULTS.results
            break
        except Exception:
            if attempt == 1:
                # device unavailable/wedged -> slow but exact host path
                return _host_reference(inputs, true_labels, prev_counts, tail_mask)

    # [P, n_chunks] per core -> per-tile sums [P, T]; row = core*RPC + t*P + p
    plans = _chunk_plans()
    offs = np.cumsum([0] + [len(p) for p in plans])[:-1]
    S = np.concatenate(
        [np.add.reduceat(r["s_out"].astype(np.float64), offs, axis=1).T.reshape(-1)
         for r in res]
    )
    # cnt_out is [P, NTAIL, T] raw 0/1 masks
    cnt_tail = np.sum(
        [r["cnt_out"].sum(axis=(0, 2)) for r in res], axis=0
    ).astype(np.float64)

    xt = inputs[np.arange(B), true_labels].astype(np.float64)
    p = np.exp(xt - np.log(S))
    base = -np.log(p + 1e-7) * (1.0 - p)

    is_tail = tail_mask[true_labels]
    prev = prev_counts[true_labels].astype(np.float64)
    curr = np.zeros(B, dtype=np.float64)
    if is_tail.any():
        curr[is_tail] = cnt_tail[true_labels[is_tail] - (C - NTAIL)]
    tail_w = np.where((prev > 0) & (curr < prev), 4.0,
                      np.where((prev > 0) & (curr > prev), 2.0, 3.0))
    w = np.where(is_tail, tail_w, 1.0)

    return np.array((base * w).mean() * 0.1, dtype=np.float32)


# revision 28
# speedup vs baseline: 1.1369x; 1.1368x over previous
"""Trainium2 Bass kernel for nn_AdditionalTermLayer (focal/tail-weighted CE penalty).

Strategy (data-parallel over the batch, 8 NeuronCores):
  - Each core streams its [2048, 8192] f32 shard of `inputs` through SBUF once,
    in 128-row tiles, each loaded as NSPLIT column chunks so DVE/ACT compute on
    chunk j overlaps the DMA of chunk j+1 (DMA is the roofline: ~64 MiB/core).
  - Per tile: row max (DVE chunk reduces + combine), sum(exp(x)) (ACT exp with
    accum_out; no max shift needed — the host guard bounds |x|), and per-row
    `x[:, C-16:] >= rowmax` compares (DVE) whose sums give the argmax-count
    histogram restricted to the 16 tail classes (the only classes whose counts
    the loss actually needs — this avoids computing argmax positions entirely).
  - Host combines the tiny per-core outputs: gathers x[b, label], computes the
    per-sample penalty, the tail-class histogram all-reduce, adaptive weights,
    and the final scalar mean.
"""

import sys
import types

import numpy as np


def _ensure_ntff_hook():
    """The axon boot registers its NTFF profile hook only if
    `antenv.axon_hooks` exists; on images where it doesn't, bass_utils
    crashes importing it under BASS_TRACE. Provide the module and register
    the ctypes-based hook ourselves so profiling works."""
    try:
        import antenv.axon_hooks  # noqa: F401
        return
    except ImportError:
        pass
    mod = types.ModuleType("antenv.axon_hooks")
    mod._hook = None

    def set_axon_ntff_profile_hook(h):
        mod._hook = h

    def get_axon_ntff_profile_hook():
        return mod._hook

    mod.set_axon_ntff_profile_hook = set_axon_ntff_profile_hook
    mod.get_axon_ntff_profile_hook = get_axon_ntff_profile_hook
    sys.modules["antenv.axon_hooks"] = mod
    try:
        import antenv
        antenv.axon_hooks = mod
    except ImportError:
        pass
    try:
        from trn_agent_boot.trn_boot import _ntff_profile_via_ctypes
        hook = _ntff_profile_via_ctypes("/opt/axon/libaxon_pjrt.so")
        if hook is not None:
            set_axon_ntff_profile_hook(hook)
    except Exception:
        pass


_ensure_ntff_hook()

import concourse.tile as tile
from concourse import bacc, mybir
from concourse.bass_utils import run_bass_kernel_spmd

B = 16384
C = 8192
N_CORES = 8
RPC = B // N_CORES  # rows per core = 2048
P = 128             # SBUF partitions
T = RPC // P        # tiles per core = 16
NTAIL = 16          # classes whose argmax-counts we need (last 16)

F32 = mybir.dt.float32
NSPLIT = 4       # column chunks per tile (pipeline granularity)
TAPER = False    # shrink the last tile's trailing chunks to cut the drain tail
DUAL_RING_TILES = 0 # tiles whose chunk DMAs alternate Sync/Scalar HWDGE rings


def _chunk_plans(nsplit=NSPLIT, taper=TAPER):
    W = C // nsplit
    plans = [[W] * nsplit for _ in range(T)]
    if taper:
        plans[T - 1] = [2048, 2048, 2048, 1920, 128]
    assert all(sum(p) == C for p in plans)
    return plans

_COMPILED_NC = None
LAST_RESULTS = None  # test harness reads exec_time_ns from here


def _build_nc(nsplit=NSPLIT, taper=TAPER, dual_ring_tiles=DUAL_RING_TILES):
    nc = bacc.Bacc(
        "TRN2",
        target_bir_lowering=False,
        debug=False,
        num_devices=N_CORES,
    )
    plans = _chunk_plans(nsplit, taper)
    n_chunks = sum(len(p) for p in plans)
    max_w = max(w for p in plans for w in p)
    x_ext = nc.dram_tensor("x", [RPC, C], F32, kind="ExternalInput")
    s_ext = nc.dram_tensor("s_out", [P, n_chunks], F32, kind="ExternalOutput")
    cnt_ext = nc.dram_tensor("cnt_out", [P, NTAIL, T], F32, kind="ExternalOutput")

    with tile.TileContext(nc) as tc:
        with (
            tc.tile_pool(name="xin", bufs=5) as xin_pool,
            tc.tile_pool(name="stats", bufs=1) as stats_pool,
            tc.tile_pool(name="dump", bufs=1) as dump_pool,
        ):
            m_all = stats_pool.tile([P, T], F32, tag="m_all")
            s_all = stats_pool.tile([P, n_chunks], F32, tag="s_all")
            ge_all = stats_pool.tile([P, NTAIL, T], F32, tag="ge_all")
            dump = dump_pool.tile([P, max_w], F32, tag="dump")

            ci = 0  # global chunk index
            for t, plan in enumerate(plans):
                xt = xin_pool.tile([P, C], F32, tag="xt")
                rows = slice(t * P, (t + 1) * P)
                mh = xin_pool.tile([P, len(plan)], F32, tag="mh")
                col = 0
                for j, w in enumerate(plan):
                    cols = slice(col, col + w)
                    col += w
                    # chunked DMAs so compute on chunk j overlaps the load of
                    # j+1; during ramp-up alternate the two HWDGE rings
                    dma_eng = (
                        nc.scalar if (t < dual_ring_tiles and j % 2) else nc.sync
                    )
                    dma_eng.dma_start(out=xt[:, cols], in_=x_ext[rows, cols])
                    # chunk max (feeds only the tail-class compares)
                    nc.vector.tensor_reduce(
                        out=mh[:, j:j + 1], in_=xt[:, cols],
                        axis=mybir.AxisListType.X, op=mybir.AluOpType.max,
                    )
                    # sum(exp(x)) via ACT accumulate; elementwise out is
                    # discarded. No max subtraction: |x| <= 60 is guaranteed by
                    # the host-side guard, so exp can't overflow f32.
                    nc.scalar.activation(
                        out=dump[:, 0:w],
                        in_=xt[:, cols],
                        func=mybir.ActivationFunctionType.Exp,
                        bias=0.0,
                        scale=1.0,
                        accum_out=s_all[:, ci:ci + 1],
                    )
                    ci += 1
                nc.vector.tensor_reduce(
                    out=m_all[:, t:t + 1], in_=mh[:],
                    axis=mybir.AxisListType.X, op=mybir.AluOpType.max,
                )

                # tail-class hit mask: x[:, C-16:] >= rowmax  (1.0 iff argmax)
                nc.vector.tensor_scalar(
                    ge_all[:, :, t:t + 1],
                    xt[:, C - NTAIL:C],
                    m_all[:, t:t + 1],
                    None,
                    mybir.AluOpType.is_ge,
                )

            # raw per-tile masks go to the host (it sums them); no device
            # reduction in the drain-critical path
            nc.sync.dma_start(out=s_ext[:, :], in_=s_all[:])
            nc.sync.dma_start(out=cnt_ext[:, :, :], in_=ge_all[:])

    nc.compile()
    return nc


def _get_nc():
    global _COMPILED_NC
    if _COMPILED_NC is None:
        _COMPILED_NC = _build_nc()
    return _COMPILED_NC


def _host_reference(x, true_labels, prev_counts, tail_mask):
    """Pure-numpy fallback mirroring the reference; used only if tail_mask is
    not a subset of the last NTAIL classes (never expected for this problem)."""
    preds = np.argmax(x, axis=-1)
    curr_counts = np.bincount(preds, minlength=x.shape[1]).astype(np.float64)
    m = x.max(axis=-1)
    S = np.exp(x - m[:, None]).sum(axis=-1)
    xt = x[np.arange(x.shape[0]), true_labels]
    p = np.exp(xt - m - np.log(S))
    base = -np.log(p + 1e-7) * (1.0 - p)
    prev = prev_counts[true_labels].astype(np.float64)
    curr = curr_counts[true_labels]
    tail_w = np.where((prev > 0) & (curr < prev), 4.0,
                      np.where((prev > 0) & (curr > prev), 2.0, 3.0))
    w = np.where(tail_mask[true_labels], tail_w, 1.0)
    return np.array((base * w).mean() * 0.1, dtype=np.float32)


def kernel(inputs, true_labels, prev_counts, tail_mask):
    global LAST_RESULTS
    inputs = np.asarray(inputs, dtype=np.float32)
    true_labels = np.asarray(true_labels).astype(np.int64)
    prev_counts = np.asarray(prev_counts)
    tail_mask = np.asarray(tail_mask).astype(bool)
    assert inputs.shape == (B, C), inputs.shape

    if not np.isfinite(inputs).all():
        inputs = np.nan_to_num(inputs)

    tail_idx = np.flatnonzero(tail_mask)
    if (tail_idx.size and tail_idx.min() < C - NTAIL) or np.abs(inputs).max() > 60.0:
        # unexpected tail layout, or values large enough that the device's
        # unshifted exp could overflow -> use the exact host path
        return _host_reference(inputs, true_labels, prev_counts, tail_mask)

    in_maps = [{"x": inputs[i * RPC:(i + 1) * RPC]} for i in range(N_CORES)]
    res = None
    for attempt in range(2):
        try:
            nc = _get_nc()
            LAST_RESULTS = run_bass_kernel_spmd(
                nc, in_maps, core_ids=list(range(N_CORES))
            )
            res = LAST_RESULTS.results
            break
        except Exception:
            if attempt == 1:
                # device unavailable/wedged -> slow but exact host path
                return _host_reference(inputs, true_labels, prev_counts, tail_mask)

    # [P, n_chunks] per core -> per-tile sums [P, T]; row = core*RPC + t*P + p
    plans = _chunk_plans()
    offs = np.cumsum([0] + [len(p) for p in plans])[:-1]
    S = np.concatenate(
        [np.add.reduceat(r["s_out"].astype(np.float64), offs, axis=1).T.reshape(-1)
         for r in res]
    )
    # cnt_out is [P, NTAIL, T] raw 0/1 masks
    cnt_tail = np.sum(
        [r["cnt_out"].sum(axis=(0, 2)) for r in res], axis=0
    ).astype(np.float64)

    xt = inputs[np.arange(B), true_labels].astype(np.float64)
    p = np.exp(xt - np.log(S))
    base = -np.log(p + 1e-7) * (1.0 - p)

    is_tail = tail_mask[true_labels]
    prev = prev_counts[true_labels].astype(np.float64)
    curr = np.zeros(B, dtype=np.float64)
    if is_tail.any():
        curr[is_tail] = cnt_tail[true_labels[is_tail] - (C - NTAIL)]
    tail_w = np.where((prev > 0) & (curr < prev), 4.0,
                      np.where((prev > 0) & (curr > prev), 2.0, 3.0))
    w = np.where(is_tail, tail_w, 1.0)

    return np.array((base * w).mean() * 0.1, dtype=np.float32)
